# revision 23
# baseline (speedup 1.0000x reference)
"""CQAttention (BiDAF context-query attention) forward kernel for 8 Trainium2
NeuronCores.

Full inputs: context (64,128,1024) f32, question (64,128,128) f32, w (384,) f32.
Full output: (64, 512, 1024) f32.

Sharding: pure data parallel over batch — 8 batches per core, w replicated.

Math (per batch, X = context[b] (H,C), Y = question[b] (H,Q), w=(wq,wc,wcq)):
    S^T = (wcq*Y + wc 1^T)^T @ X              # (Q,C); wq term is softmax-invariant
    P   = exp(S^T)                            # unnormalized softmax numerators
    r   = 1/rowsum(P)                         # softmax denominators (per q-row)
    A   = (diag(r) Y^T)^T @ P                 # = a^T                (H,C)
    tt  = P @ X^T                             # (Q,H) via PE transposes of P,X
    Bm  = (diag(r^2) tt)^T @ P                # = b^T = (s1 (s1^T c))^T  (H,C)
    out = [X; A; X*A; X*Bm]                   # (4H, C)

The whole pipeline runs in bf16 (inputs cast host-side; matmuls accumulate in
f32 PSUM; exp reads f32 scores, row-sums accumulate in f32); the three
computed output blocks are stored bf16 and upcast host-side; block 0
(== context) is filled host-side from the exact f32 input. Max-normalized
relative error ~2.4e-3 vs the 2e-2 gate.

Engine split per batch: PE does 2 score MMs, 16 transposes, 8 tt MMs, 2+2
A/B MMs; ACT does the 2 exp chunks + X^T evac + half the A evac; DVE does
P^T evac, the other A half, X*B (straight from PSUM — B is never evacuated),
tts (r^2 fused as a double-scalar multiply), softmax stats; GpSimd does X*A.
S, A and B rotate through one 3-buf [128,1024]-f32 PSUM pool so scores for
batch b+1 never wait on batch b's evacuations.
"""

import os
import sys

import numpy as np

if "/opt/trn_rl_repo" not in sys.path:
    sys.path.insert(0, "/opt/trn_rl_repo")

B, H, C, Q = 64, 128, 1024, 128
NCORES = 8
BPC = B // NCORES  # batches per core


def _ensure_ntff_hook():
    """This container's `antenv` stub lacks `axon_hooks`, which
    bass_utils needs for NTFF profiling under axon (trace=True). Install
    a functional shadow module + register the ctypes-based hook."""
    import types

    try:
        from antenv.axon_hooks import get_axon_ntff_profile_hook  # noqa: F401

        return  # real module present
    except ImportError:
        pass
    try:
        import antenv

        mod = types.ModuleType("antenv.axon_hooks")
        _state = {"hook": None}

        def set_axon_ntff_profile_hook(h):
            _state["hook"] = h

        def get_axon_ntff_profile_hook():
            return _state["hook"]

        mod.set_axon_ntff_profile_hook = set_axon_ntff_profile_hook
        mod.get_axon_ntff_profile_hook = get_axon_ntff_profile_hook
        sys.modules["antenv.axon_hooks"] = mod
        antenv.axon_hooks = mod

        from trn_agent_boot.trn_boot import _ntff_profile_via_ctypes

        set_axon_ntff_profile_hook(
            _ntff_profile_via_ctypes("/opt/axon/libaxon_pjrt.so")
        )
    except Exception:
        pass  # profiling degrades; compute still works


_ensure_ntff_hook()

LAST_RESULTS = None
_NC = None


def _build():
    from contextlib import ExitStack

    import concourse.bacc as bacc
    import concourse.mybir as mybir
    import concourse.tile as tile
    from concourse import masks

    f32 = mybir.dt.float32
    f32r = mybir.dt.float32r
    bf16 = mybir.dt.bfloat16
    EXP = mybir.ActivationFunctionType.Exp
    COPY = mybir.ActivationFunctionType.Copy
    MULT = mybir.AluOpType.mult
    ADD = mybir.AluOpType.add
    DIV = mybir.AluOpType.divide

    nc = bacc.Bacc(
        "TRN2", target_bir_lowering=False, debug=False, enable_asserts=False
    )
    ctx_t = nc.dram_tensor("context", (BPC, H, C), bf16, kind="ExternalInput").ap()
    q_t = nc.dram_tensor("question", (BPC, H, Q), bf16, kind="ExternalInput").ap()
    w_t = nc.dram_tensor("w", (3 * H,), f32, kind="ExternalInput").ap()
    out_t = nc.dram_tensor("out", (BPC, 3, H, C), bf16, kind="ExternalOutput").ap()

    with tile.TileContext(nc) as tc, ExitStack() as ctx:
        const = ctx.enter_context(tc.tile_pool(name="const", bufs=1))
        sb = ctx.enter_context(tc.tile_pool(name="sb", bufs=3))
        sbx = ctx.enter_context(tc.tile_pool(name="sbx", bufs=2))
        sb3 = ctx.enter_context(tc.tile_pool(name="sb3", bufs=3))
        # PSUM (8 banks): ps_big 3x[128,1024]f32 (6 banks; S, A, B rotate),
        # ps_tp 2x (X^T / P^T bf16 single-bank tiles and the small f32 tt).
        ps_big = ctx.enter_context(tc.tile_pool(name="ps_big", bufs=3, space="PSUM"))
        ps_tp = ctx.enter_context(tc.tile_pool(name="ps_tp", bufs=2, space="PSUM"))

        # input DMAs first so nothing delays them in the queues
        Xpair0 = sbx.tile([H, 2 * C], bf16, tag="Xp")
        nc.sync.dma_start(
            Xpair0[:].rearrange("p (k c) -> p k c", k=2),
            ctx_t[0:2].transpose([1, 0, 2]),
        )
        Yall = const.tile([128, BPC * Q], bf16, tag="Yall")
        nc.sync.dma_start(
            Yall[:].rearrange("p (b q) -> p b q", b=BPC),
            q_t.transpose([1, 0, 2]),
        )
        w_row = const.tile([1, 3 * H], f32r, tag="w_row")
        nc.sync.dma_start(w_row[:], w_t.unsqueeze(0).bitcast(f32r))

        ident = const.tile([128, 128], f32, tag="ident")
        masks.make_identity(nc, ident[:])
        identb = const.tile([128, 128], bf16, tag="identb")
        nc.vector.tensor_copy(identb[:], ident[:])
        identr = const.tile([128, 128], f32r, tag="identr")
        nc.gpsimd.tensor_copy(identr[:], ident[:])
        ones = const.tile([128, 1], f32, tag="ones")
        nc.gpsimd.memset(ones[:], 1.0)

        # w columns via K=1 PE matmuls against identity
        wps = ps_big.tile([128, C], f32, tag="big")
        nc.tensor.matmul(
            wps[:, 0:128], w_row[0:1, H : 2 * H], identr[0:1, 0:128],
            start=True, stop=True,
        )
        nc.tensor.matmul(
            wps[:, 128:256], w_row[0:1, 2 * H : 3 * H], identr[0:1, 0:128],
            start=True, stop=True,
        )
        wc = const.tile([128, 1], f32, tag="wc")
        wcq = const.tile([128, 1], f32, tag="wcq")
        nc.vector.tensor_copy(wc[:], wps[:, 0:1])
        nc.vector.tensor_copy(wcq[:], wps[:, 128:129])

        # all Z = wcq*Y + wc in one op; all Y^T via 8 PE transposes
        Zall = const.tile([128, BPC * Q], bf16, tag="Zall")
        nc.vector.tensor_scalar(Zall[:], Yall[:], wcq[:], wc[:], MULT, ADD)
        YTall = const.tile([128, BPC * H], bf16, tag="YTall")
        ytp = ps_tp.tile([128, BPC * Q], bf16, tag="tp")
        for bb in range(BPC):
            nc.tensor.transpose(
                ytp[:, bb * Q : (bb + 1) * Q], Yall[:, bb * Q : (bb + 1) * Q],
                identb[:],
            )
        nc.vector.tensor_copy(YTall[:], ytp[:])

        st = {}  # per-batch live tiles (3-stage pipeline keeps ~3 alive)

        pairs = {0: Xpair0}

        def front(b):
            state = st[b] = {}
            # prefetch the next X pair one batch ahead of first use
            if b % 2 == 1 and b + 1 < BPC:
                g = (b + 1) // 2
                Xpair = sbx.tile([H, 2 * C], bf16, tag="Xp")
                nc.sync.dma_start(
                    Xpair[:].rearrange("p (k c) -> p k c", k=2),
                    ctx_t[2 * g : 2 * g + 2].transpose([1, 0, 2]),
                )
                pairs[g] = Xpair
                pairs.pop(g - 2, None)
            X = pairs[b // 2][:, (b % 2) * C : (b % 2 + 1) * C]

            # scores (2 matmuls into one 2-bank tile) + 2 exp chunks
            Sps = ps_big.tile([Q, C], f32, tag="big")
            P = sb.tile([Q, C], bf16, tag="P")
            dh = sb.tile([Q, 2], f32, tag="dh")
            Zb = Zall[:, b * Q : (b + 1) * Q]
            for j in range(2):
                nc.tensor.matmul(
                    Sps[:, j * 512 : (j + 1) * 512],
                    Zb,
                    X[:, j * 512 : (j + 1) * 512],
                    start=True,
                    stop=True,
                )
                nc.scalar.activation(
                    P[:, j * 512 : (j + 1) * 512],
                    Sps[:, j * 512 : (j + 1) * 512],
                    EXP,
                    accum_out=dh[:, j : j + 1],
                )
            state.update(X=X, P=P, dh=dh, b=b)

        def stats(b):
            # softmax stats; emitted late so the DVE queue serves the
            # data-ready evacuations first (these only gate A of batch b,
            # which runs next cycle)
            state = st[b]
            dh = state["dh"]
            dsum = sb.tile([Q, 1], f32, tag="dsum")
            nc.vector.tensor_add(dsum[:], dh[:, 0:1], dh[:, 1:2])
            rr = sb.tile([Q, 1], f32, tag="rr")
            nc.vector.reciprocal(rr[:], dsum[:])
            YTs = sb.tile([Q, H], bf16, tag="YTs")
            nc.vector.tensor_scalar_mul(
                YTs[:], YTall[:, b * H : (b + 1) * H], rr[:]
            )
            state.update(rr=rr, YTs=YTs)

        def mid(b):
            state = st[b]
            X, P, YTs, rr = state["X"], state["P"], state["YTs"], state["rr"]
            # X^T / P^T: 8 transposes each into the single-bank bf16 tile,
            # evacuated with one copy (ACT for X^T, DVE for P^T).
            xtp = ps_tp.tile([128, C], bf16, tag="tp")
            for k in range(8):
                nc.tensor.transpose(
                    xtp[:, k * 128 : (k + 1) * 128],
                    X[:, k * 128 : (k + 1) * 128],
                    identb[:],
                )
            XT = sb.tile([128, C], bf16, tag="XT")
            nc.vector.tensor_copy(XT[:], xtp[:])

            ptp = ps_tp.tile([128, C], bf16, tag="tp")
            for k in range(8):
                nc.tensor.transpose(
                    ptp[:, k * 128 : (k + 1) * 128],
                    P[:, k * 128 : (k + 1) * 128],
                    identb[:],
                )
            PT = sb.tile([128, C], bf16, tag="PT")
            nc.vector.tensor_copy(PT[:], ptp[:])

            # tt = P @ X^T, accumulated over the 8 c-chunks
            tt = ps_tp.tile([Q, H], f32, tag="tp")
            for k in range(8):
                nc.tensor.matmul(
                    tt[:],
                    PT[:, k * 128 : (k + 1) * 128],
                    XT[:, k * 128 : (k + 1) * 128],
                    start=(k == 0),
                    stop=(k == 7),
                )
            # tts = tt * r^2, r^2 fused as a double scalar multiply
            tts = sb.tile([Q, H], bf16, tag="tts")
            nc.vector.tensor_scalar(tts[:], tt[:], rr[:], rr[:], MULT, MULT)

            # A matmuls into one 2-bank tile; evac halves on ACT and DVE.
            S3 = sb3.tile([H, 3 * C], bf16, tag="S3")
            Aps = ps_big.tile([H, C], f32, tag="big")
            for j in range(2):
                nc.tensor.matmul(
                    Aps[:, j * 512 : (j + 1) * 512],
                    YTs[:],
                    P[:, j * 512 : (j + 1) * 512],
                    start=True,
                    stop=True,
                )
            nc.scalar.activation(S3[:, 0:512], Aps[:, 0:512], COPY)
            nc.scalar.activation(S3[:, 512:1024], Aps[:, 512:1024], COPY)
            state.update(tts=tts, S3=S3)

        def back(b):
            state = st[b]
            X, P, tts, S3 = state["X"], state["P"], state["tts"], state["S3"]
            last = b == BPC - 1
            if last:
                nc.sync.dma_start(out_t[b, 0], S3[:, 0:C])
                for j in range(2):
                    nc.gpsimd.tensor_mul(
                        S3[:, C + j * 512 : C + (j + 1) * 512],
                        X[:, j * 512 : (j + 1) * 512],
                        S3[:, j * 512 : (j + 1) * 512],
                    )
                    nc.sync.dma_start(
                        out_t[b, 1, :, j * 512 : (j + 1) * 512],
                        S3[:, C + j * 512 : C + (j + 1) * 512],
                    )
            else:
                nc.gpsimd.tensor_mul(S3[:, C : 2 * C], X[:], S3[:, 0:C])
            # B matmuls into one 2-bank tile; X*B straight from PSUM (one op)
            Bps = ps_big.tile([H, C], f32, tag="big")
            for j in range(2):
                nc.tensor.matmul(
                    Bps[:, j * 512 : (j + 1) * 512],
                    tts[:],
                    P[:, j * 512 : (j + 1) * 512],
                    start=True,
                    stop=True,
                )
            if last:
                for j in range(2):
                    nc.vector.tensor_mul(
                        S3[:, 2 * C + j * 512 : 2 * C + (j + 1) * 512],
                        X[:, j * 512 : (j + 1) * 512],
                        Bps[:, j * 512 : (j + 1) * 512],
                    )
                    nc.sync.dma_start(
                        out_t[b, 2, :, j * 512 : (j + 1) * 512],
                        S3[:, 2 * C + j * 512 : 2 * C + (j + 1) * 512],
                    )
            else:
                nc.vector.tensor_mul(S3[:, 2 * C : 3 * C], X[:], Bps[:])
                nc.sync.dma_start(
                    out_t[b].transpose([1, 0, 2]),
                    S3[:].rearrange("p (k c) -> p k c", k=3),
                )

        # 3-stage software pipeline: cycle i emits front(i) | mid(i-1) |
        # back(i-2). Each batch's serial dependency chain (exp -> stats ->
        # evacs -> tt -> B -> X*B -> DMA) gets ~2 cycles of latency budget,
        # so the steady-state cycle is set by engine throughput instead.
        for i in range(BPC + 2):
            if i < BPC:
                front(i)
            if 0 <= i - 1 < BPC:
                mid(i - 1)
            if 0 <= i - 2 < BPC:
                back(i - 2)
                del st[i - 2]
            if i < BPC:
                stats(i)

    nc.compile()
    return nc


def kernel(context, question, w):
    global _NC, LAST_RESULTS
    import ml_dtypes

    from concourse import bass_utils

    if _NC is None:
        _NC = _build()

    context = np.ascontiguousarray(np.asarray(context), dtype=np.float32)
    question = np.ascontiguousarray(np.asarray(question), dtype=np.float32)
    w = np.ascontiguousarray(np.asarray(w), dtype=np.float32)
    ctx_bf = context.astype(ml_dtypes.bfloat16)
    q_bf = question.astype(ml_dtypes.bfloat16)

    in_maps = [
        {
            "context": ctx_bf[c * BPC : (c + 1) * BPC],
            "question": q_bf[c * BPC : (c + 1) * BPC],
            "w": w,
        }
        for c in range(NCORES)
    ]
    trace = bool(int(os.environ.get("KTRACE", "0")))
    LAST_RESULTS = bass_utils.run_bass_kernel_spmd(
        _NC, in_maps, core_ids=list(range(NCORES)), trace=trace
    )
    out = np.empty((B, 4 * H, C), dtype=np.float32)
    out[:, 0:H, :] = context
    for c in range(NCORES):
        blk = np.asarray(LAST_RESULTS.results[c]["out"]).astype(np.float32)
        out[c * BPC : (c + 1) * BPC, H:, :] = blk.reshape(BPC, 3 * H, C)
    return out


# revision 24
# speedup vs baseline: 1.0461x; 1.0461x over previous
"""CQAttention (BiDAF context-query attention) forward kernel for 8 Trainium2
NeuronCores.

Full inputs: context (64,128,1024) f32, question (64,128,128) f32, w (384,) f32.
Full output: (64, 512, 1024) f32.

Sharding: pure data parallel over batch — 8 batches per core, w replicated.

Math (per batch, X = context[b] (H,C), Y = question[b] (H,Q), w=(wq,wc,wcq)):
    S^T = (wcq*Y + wc 1^T)^T @ X              # (Q,C); wq term is softmax-invariant
    P   = exp(S^T)                            # unnormalized softmax numerators
    r   = 1/rowsum(P)                         # softmax denominators (per q-row)
    A   = (diag(r) Y^T)^T @ P                 # = a^T                (H,C)
    tt  = P @ X^T                             # (Q,H) via PE transposes of P,X
    Bm  = (diag(r^2) tt)^T @ P                # = b^T = (s1 (s1^T c))^T  (H,C)
    out = [X; A; X*A; X*Bm]                   # (4H, C)

The whole pipeline runs in bf16 (inputs cast host-side; matmuls accumulate in
f32 PSUM; exp reads f32 scores, row-sums accumulate in f32); the three
computed output blocks are stored bf16 and upcast host-side; block 0
(== context) is filled host-side from the exact f32 input. Max-normalized
relative error ~2.4e-3 vs the 2e-2 gate.

Engine split per batch: PE does 2 score MMs, 16 transposes, 8 tt MMs, 2+2
A/B MMs; ACT does the 2 exp chunks + X^T evac + half the A evac; DVE does
P^T evac, the other A half, X*B (straight from PSUM — B is never evacuated),
tts (r^2 fused as a double-scalar multiply), softmax stats; GpSimd does X*A.
S, A and B rotate through one 3-buf [128,1024]-f32 PSUM pool so scores for
batch b+1 never wait on batch b's evacuations.
"""

import os
import sys

import numpy as np

if "/opt/trn_rl_repo" not in sys.path:
    sys.path.insert(0, "/opt/trn_rl_repo")

B, H, C, Q = 64, 128, 1024, 128
NCORES = 8
BPC = B // NCORES  # batches per core


def _ensure_ntff_hook():
    """This container's `antenv` stub lacks `axon_hooks`, which
    bass_utils needs for NTFF profiling under axon (trace=True). Install
    a functional shadow module + register the ctypes-based hook."""
    import types

    try:
        from antenv.axon_hooks import get_axon_ntff_profile_hook  # noqa: F401

        return  # real module present
    except ImportError:
        pass
    try:
        import antenv

        mod = types.ModuleType("antenv.axon_hooks")
        _state = {"hook": None}

        def set_axon_ntff_profile_hook(h):
            _state["hook"] = h

        def get_axon_ntff_profile_hook():
            return _state["hook"]

        mod.set_axon_ntff_profile_hook = set_axon_ntff_profile_hook
        mod.get_axon_ntff_profile_hook = get_axon_ntff_profile_hook
        sys.modules["antenv.axon_hooks"] = mod
        antenv.axon_hooks = mod

        from trn_agent_boot.trn_boot import _ntff_profile_via_ctypes

        set_axon_ntff_profile_hook(
            _ntff_profile_via_ctypes("/opt/axon/libaxon_pjrt.so")
        )
    except Exception:
        pass  # profiling degrades; compute still works


_ensure_ntff_hook()

LAST_RESULTS = None
_NC = None


def _build():
    from contextlib import ExitStack

    import concourse.bacc as bacc
    import concourse.mybir as mybir
    import concourse.tile as tile
    from concourse import masks

    f32 = mybir.dt.float32
    f32r = mybir.dt.float32r
    bf16 = mybir.dt.bfloat16
    EXP = mybir.ActivationFunctionType.Exp
    COPY = mybir.ActivationFunctionType.Copy
    MULT = mybir.AluOpType.mult
    ADD = mybir.AluOpType.add
    DIV = mybir.AluOpType.divide

    nc = bacc.Bacc(
        "TRN2", target_bir_lowering=False, debug=False, enable_asserts=False
    )
    ctx_t = nc.dram_tensor("context", (BPC, H, C), bf16, kind="ExternalInput").ap()
    q_t = nc.dram_tensor("question", (BPC, H, Q), bf16, kind="ExternalInput").ap()
    w_t = nc.dram_tensor("w", (3 * H,), f32, kind="ExternalInput").ap()
    out_t = nc.dram_tensor("out", (BPC, 3, H, C), bf16, kind="ExternalOutput").ap()

    with tile.TileContext(nc) as tc, ExitStack() as ctx:
        const = ctx.enter_context(tc.tile_pool(name="const", bufs=1))
        sb = ctx.enter_context(tc.tile_pool(name="sb", bufs=3))
        sbx = ctx.enter_context(tc.tile_pool(name="sbx", bufs=2))
        sb3 = ctx.enter_context(tc.tile_pool(name="sb3", bufs=3))
        # PSUM (8 banks): ps_big 3x[128,1024]f32 (6 banks; S, A, B rotate),
        # ps_tp 2x (X^T / P^T bf16 single-bank tiles and the small f32 tt).
        ps_big = ctx.enter_context(tc.tile_pool(name="ps_big", bufs=3, space="PSUM"))
        ps_tp = ctx.enter_context(tc.tile_pool(name="ps_tp", bufs=2, space="PSUM"))

        # input DMAs first so nothing delays them in the queues
        Xpair0 = sbx.tile([H, 2 * C], bf16, tag="Xp")
        nc.sync.dma_start(
            Xpair0[:].rearrange("p (k c) -> p k c", k=2),
            ctx_t[0:2].transpose([1, 0, 2]),
        )
        Yall = const.tile([128, BPC * Q], bf16, tag="Yall")
        nc.sync.dma_start(
            Yall[:].rearrange("p (b q) -> p b q", b=BPC),
            q_t.transpose([1, 0, 2]),
        )
        w_row = const.tile([1, 3 * H], f32r, tag="w_row")
        nc.sync.dma_start(w_row[:], w_t.unsqueeze(0).bitcast(f32r))

        ident = const.tile([128, 128], f32, tag="ident")
        masks.make_identity(nc, ident[:])
        identb = const.tile([128, 128], bf16, tag="identb")
        nc.vector.tensor_copy(identb[:], ident[:])
        identr = const.tile([128, 128], f32r, tag="identr")
        nc.gpsimd.tensor_copy(identr[:], ident[:])
        ones = const.tile([128, 1], f32, tag="ones")
        nc.gpsimd.memset(ones[:], 1.0)

        # w columns via K=1 PE matmuls against identity
        wps = ps_big.tile([128, C], f32, tag="big")
        nc.tensor.matmul(
            wps[:, 0:128], w_row[0:1, H : 2 * H], identr[0:1, 0:128],
            start=True, stop=True,
        )
        nc.tensor.matmul(
            wps[:, 128:256], w_row[0:1, 2 * H : 3 * H], identr[0:1, 0:128],
            start=True, stop=True,
        )
        wc = const.tile([128, 1], f32, tag="wc")
        wcq = const.tile([128, 1], f32, tag="wcq")
        nc.vector.tensor_copy(wc[:], wps[:, 0:1])
        nc.vector.tensor_copy(wcq[:], wps[:, 128:129])

        # all Z = wcq*Y + wc in one op; all Y^T via 8 PE transposes
        Zall = const.tile([128, BPC * Q], bf16, tag="Zall")
        nc.vector.tensor_scalar(Zall[:], Yall[:], wcq[:], wc[:], MULT, ADD)
        YTall = const.tile([128, BPC * H], bf16, tag="YTall")
        ytp = ps_tp.tile([128, BPC * Q], bf16, tag="tp")
        for bb in range(BPC):
            nc.tensor.transpose(
                ytp[:, bb * Q : (bb + 1) * Q], Yall[:, bb * Q : (bb + 1) * Q],
                identb[:],
            )
        nc.vector.tensor_copy(YTall[:], ytp[:])

        st = {}  # per-batch live tiles (3-stage pipeline keeps ~3 alive)

        pairs = {0: Xpair0}

        def front(b):
            state = st[b] = {}
            # prefetch the next X pair one batch ahead of first use
            if b % 2 == 1 and b + 1 < BPC:
                g = (b + 1) // 2
                Xpair = sbx.tile([H, 2 * C], bf16, tag="Xp")
                nc.sync.dma_start(
                    Xpair[:].rearrange("p (k c) -> p k c", k=2),
                    ctx_t[2 * g : 2 * g + 2].transpose([1, 0, 2]),
                )
                pairs[g] = Xpair
                pairs.pop(g - 2, None)
            X = pairs[b // 2][:, (b % 2) * C : (b % 2 + 1) * C]

            # scores (2 matmuls into one 2-bank tile) + 2 exp chunks
            Sps = ps_big.tile([Q, C], f32, tag="big")
            P = sb.tile([Q, C], bf16, tag="P")
            dh = sb.tile([Q, 2], f32, tag="dh")
            Zb = Zall[:, b * Q : (b + 1) * Q]
            for j in range(2):
                nc.tensor.matmul(
                    Sps[:, j * 512 : (j + 1) * 512],
                    Zb,
                    X[:, j * 512 : (j + 1) * 512],
                    start=True,
                    stop=True,
                )
                nc.scalar.activation(
                    P[:, j * 512 : (j + 1) * 512],
                    Sps[:, j * 512 : (j + 1) * 512],
                    EXP,
                    accum_out=dh[:, j : j + 1],
                )
            state.update(X=X, P=P, dh=dh, b=b)

        def stats(b):
            # softmax stats; emitted late so the DVE queue serves the
            # data-ready evacuations first (these only gate A of batch b,
            # which runs next cycle)
            state = st[b]
            dh = state["dh"]
            dsum = sb.tile([Q, 1], f32, tag="dsum")
            nc.vector.tensor_add(dsum[:], dh[:, 0:1], dh[:, 1:2])
            rr = sb.tile([Q, 1], f32, tag="rr")
            nc.vector.reciprocal(rr[:], dsum[:])
            # YTs on ACT: a DVE tensor_scalar here enters 4x 2-port mode and
            # deadlocks against GpSimd's X*A for that instruction's duration
            YTs = sb.tile([Q, H], bf16, tag="YTs")
            nc.scalar.activation(
                YTs[:], YTall[:, b * H : (b + 1) * H], COPY, scale=rr[:]
            )
            state.update(rr=rr, YTs=YTs)

        def mid(b):
            state = st[b]
            X, P, YTs, rr = state["X"], state["P"], state["YTs"], state["rr"]
            # X^T / P^T: 8 transposes each into the single-bank bf16 tile,
            # evacuated with one copy (ACT for X^T, DVE for P^T).
            xtp = ps_tp.tile([128, C], bf16, tag="tp")
            for k in range(8):
                nc.tensor.transpose(
                    xtp[:, k * 128 : (k + 1) * 128],
                    X[:, k * 128 : (k + 1) * 128],
                    identb[:],
                )
            XT = sb.tile([128, C], bf16, tag="XT")
            nc.vector.tensor_copy(XT[:], xtp[:])

            ptp = ps_tp.tile([128, C], bf16, tag="tp")
            for k in range(8):
                nc.tensor.transpose(
                    ptp[:, k * 128 : (k + 1) * 128],
                    P[:, k * 128 : (k + 1) * 128],
                    identb[:],
                )
            PT = sb.tile([128, C], bf16, tag="PT")
            nc.vector.tensor_copy(PT[:], ptp[:])

            # tt = P @ X^T, accumulated over the 8 c-chunks
            tt = ps_tp.tile([Q, H], f32, tag="tp")
            for k in range(8):
                nc.tensor.matmul(
                    tt[:],
                    PT[:, k * 128 : (k + 1) * 128],
                    XT[:, k * 128 : (k + 1) * 128],
                    start=(k == 0),
                    stop=(k == 7),
                )
            # tts = tt * r^2, r^2 fused as a double scalar multiply
            tts = sb.tile([Q, H], bf16, tag="tts")
            nc.vector.tensor_scalar(tts[:], tt[:], rr[:], rr[:], MULT, MULT)

            # A matmuls into one 2-bank tile; evac halves on ACT and DVE.
            S3 = sb3.tile([H, 3 * C], bf16, tag="S3")
            Aps = ps_big.tile([H, C], f32, tag="big")
            for j in range(2):
                nc.tensor.matmul(
                    Aps[:, j * 512 : (j + 1) * 512],
                    YTs[:],
                    P[:, j * 512 : (j + 1) * 512],
                    start=True,
                    stop=True,
                )
            nc.scalar.activation(S3[:, 0:512], Aps[:, 0:512], COPY)
            nc.scalar.activation(S3[:, 512:1024], Aps[:, 512:1024], COPY)
            state.update(tts=tts, S3=S3)

        def back(b):
            state = st[b]
            X, P, tts, S3 = state["X"], state["P"], state["tts"], state["S3"]
            last = b == BPC - 1
            if last:
                nc.sync.dma_start(out_t[b, 0], S3[:, 0:C])
                for j in range(2):
                    nc.gpsimd.tensor_mul(
                        S3[:, C + j * 512 : C + (j + 1) * 512],
                        X[:, j * 512 : (j + 1) * 512],
                        S3[:, j * 512 : (j + 1) * 512],
                    )
                    nc.sync.dma_start(
                        out_t[b, 1, :, j * 512 : (j + 1) * 512],
                        S3[:, C + j * 512 : C + (j + 1) * 512],
                    )
            else:
                nc.gpsimd.tensor_mul(S3[:, C : 2 * C], X[:], S3[:, 0:C])
            # B matmuls into one 2-bank tile; X*B straight from PSUM (one op)
            Bps = ps_big.tile([H, C], f32, tag="big")
            for j in range(2):
                nc.tensor.matmul(
                    Bps[:, j * 512 : (j + 1) * 512],
                    tts[:],
                    P[:, j * 512 : (j + 1) * 512],
                    start=True,
                    stop=True,
                )
            if last:
                for j in range(2):
                    nc.vector.tensor_mul(
                        S3[:, 2 * C + j * 512 : 2 * C + (j + 1) * 512],
                        X[:, j * 512 : (j + 1) * 512],
                        Bps[:, j * 512 : (j + 1) * 512],
                    )
                    nc.sync.dma_start(
                        out_t[b, 2, :, j * 512 : (j + 1) * 512],
                        S3[:, 2 * C + j * 512 : 2 * C + (j + 1) * 512],
                    )
            else:
                nc.vector.tensor_mul(S3[:, 2 * C : 3 * C], X[:], Bps[:])
                nc.sync.dma_start(
                    out_t[b].transpose([1, 0, 2]),
                    S3[:].rearrange("p (k c) -> p k c", k=3),
                )

        # 3-stage software pipeline: cycle i emits front(i) | mid(i-1) |
        # back(i-2). Each batch's serial dependency chain (exp -> stats ->
        # evacs -> tt -> B -> X*B -> DMA) gets ~2 cycles of latency budget,
        # so the steady-state cycle is set by engine throughput instead.
        for i in range(BPC + 2):
            if i < BPC:
                front(i)
            if 0 <= i - 1 < BPC:
                mid(i - 1)
            if 0 <= i - 2 < BPC:
                back(i - 2)
                del st[i - 2]
            if i < BPC:
                stats(i)

    nc.compile()
    return nc


def kernel(context, question, w):
    global _NC, LAST_RESULTS
    import ml_dtypes

    from concourse import bass_utils

    if _NC is None:
        _NC = _build()

    context = np.ascontiguousarray(np.asarray(context), dtype=np.float32)
    question = np.ascontiguousarray(np.asarray(question), dtype=np.float32)
    w = np.ascontiguousarray(np.asarray(w), dtype=np.float32)
    ctx_bf = context.astype(ml_dtypes.bfloat16)
    q_bf = question.astype(ml_dtypes.bfloat16)

    in_maps = [
        {
            "context": ctx_bf[c * BPC : (c + 1) * BPC],
            "question": q_bf[c * BPC : (c + 1) * BPC],
            "w": w,
        }
        for c in range(NCORES)
    ]
    trace = bool(int(os.environ.get("KTRACE", "0")))
    LAST_RESULTS = bass_utils.run_bass_kernel_spmd(
        _NC, in_maps, core_ids=list(range(NCORES)), trace=trace
    )
    out = np.empty((B, 4 * H, C), dtype=np.float32)
    out[:, 0:H, :] = context
    for c in range(NCORES):
        blk = np.asarray(LAST_RESULTS.results[c]["out"]).astype(np.float32)
        out[c * BPC : (c + 1) * BPC, H:, :] = blk.reshape(BPC, 3 * H, C)
    return out


# revision 32
# speedup vs baseline: 1.2022x; 1.1493x over previous
"""CQAttention (BiDAF context-query attention) forward kernel for 8 Trainium2
NeuronCores.

Full inputs: context (64,128,1024) f32, question (64,128,128) f32, w (384,) f32.
Full output: (64, 512, 1024) f32.

Sharding: pure data parallel over batch — 8 batches per core, w replicated.

Math (per batch, X = context[b] (H,C), Y = question[b] (H,Q), w=(wq,wc,wcq)):
    S^T = (wcq*Y + wc 1^T)^T @ X              # (Q,C); wq term is softmax-invariant
    P   = exp(S^T)                            # unnormalized softmax numerators
    r   = 1/rowsum(P)                         # softmax denominators (per q-row)
    A   = (diag(r) Y^T)^T @ P                 # = a^T                (H,C)
    tt  = P @ X^T                             # (Q,H) via PE transposes of P,X
    Bm  = (diag(r^2) tt)^T @ P                # = b^T = (s1 (s1^T c))^T  (H,C)
    out = [X; A; X*A; X*Bm]                   # (4H, C)

The whole pipeline runs in bf16 (inputs cast host-side; matmuls accumulate in
f32 PSUM; exp reads f32 scores, row-sums accumulate in f32); the three
computed output blocks are stored bf16 and upcast host-side; block 0
(== context) is filled host-side from the exact f32 input. Max-normalized
relative error ~2.4e-3 vs the 2e-2 gate.

Engine split per batch: PE does 2 score MMs, 16 transposes, 8 tt MMs, 2+2
A/B MMs; ACT does the 2 exp chunks + X^T evac + half the A evac; DVE does
P^T evac, the other A half, X*B (straight from PSUM — B is never evacuated),
tts (r^2 fused as a double-scalar multiply), softmax stats; GpSimd does X*A.
S, A and B rotate through one 3-buf [128,1024]-f32 PSUM pool so scores for
batch b+1 never wait on batch b's evacuations.
"""

import os
import sys

import numpy as np

if "/opt/trn_rl_repo" not in sys.path:
    sys.path.insert(0, "/opt/trn_rl_repo")

B, H, C, Q = 64, 128, 1024, 128
NCORES = 8
BPC = B // NCORES  # batches per core


def _ensure_ntff_hook():
    """This container's `antenv` stub lacks `axon_hooks`, which
    bass_utils needs for NTFF profiling under axon (trace=True). Install
    a functional shadow module + register the ctypes-based hook."""
    import types

    try:
        from antenv.axon_hooks import get_axon_ntff_profile_hook  # noqa: F401

        return  # real module present
    except ImportError:
        pass
    try:
        import antenv

        mod = types.ModuleType("antenv.axon_hooks")
        _state = {"hook": None}

        def set_axon_ntff_profile_hook(h):
            _state["hook"] = h

        def get_axon_ntff_profile_hook():
            return _state["hook"]

        mod.set_axon_ntff_profile_hook = set_axon_ntff_profile_hook
        mod.get_axon_ntff_profile_hook = get_axon_ntff_profile_hook
        sys.modules["antenv.axon_hooks"] = mod
        antenv.axon_hooks = mod

        from trn_agent_boot.trn_boot import _ntff_profile_via_ctypes

        set_axon_ntff_profile_hook(
            _ntff_profile_via_ctypes("/opt/axon/libaxon_pjrt.so")
        )
    except Exception:
        pass  # profiling degrades; compute still works


_ensure_ntff_hook()

LAST_RESULTS = None
_NC = None


def _build():
    from contextlib import ExitStack

    import concourse.bacc as bacc
    import concourse.mybir as mybir
    import concourse.tile as tile
    from concourse import masks

    f32 = mybir.dt.float32
    f32r = mybir.dt.float32r
    bf16 = mybir.dt.bfloat16
    EXP = mybir.ActivationFunctionType.Exp
    COPY = mybir.ActivationFunctionType.Copy
    MULT = mybir.AluOpType.mult
    ADD = mybir.AluOpType.add
    DIV = mybir.AluOpType.divide

    nc = bacc.Bacc(
        "TRN2", target_bir_lowering=False, debug=False, enable_asserts=False
    )
    ctx_t = nc.dram_tensor("context", (BPC, H, C), bf16, kind="ExternalInput").ap()
    q_t = nc.dram_tensor("question", (BPC, H, Q), bf16, kind="ExternalInput").ap()
    w_t = nc.dram_tensor("w", (3 * H,), f32, kind="ExternalInput").ap()
    out_t = nc.dram_tensor("out", (BPC, 3, H, C), bf16, kind="ExternalOutput").ap()

    with tile.TileContext(nc) as tc, ExitStack() as ctx:
        const = ctx.enter_context(tc.tile_pool(name="const", bufs=1))
        sb = ctx.enter_context(tc.tile_pool(name="sb", bufs=3))
        sbx = ctx.enter_context(tc.tile_pool(name="sbx", bufs=3))
        sb3 = ctx.enter_context(tc.tile_pool(name="sb3", bufs=3))
        # PSUM (8 banks): ps_big 3x[128,1024]f32 (6 banks; S, A, B rotate),
        # ps_tp 2x (X^T / P^T bf16 single-bank tiles and the small f32 tt).
        ps_big = ctx.enter_context(tc.tile_pool(name="ps_big", bufs=3, space="PSUM"))
        ps_tp = ctx.enter_context(tc.tile_pool(name="ps_tp", bufs=2, space="PSUM"))

        # input DMAs first, spread across both HWDGE queues so the three
        # startup transfers run concurrently
        Xpair0 = sbx.tile([H, 2 * C], bf16, tag="Xp")
        nc.sync.dma_start(
            Xpair0[:].rearrange("p (k c) -> p k c", k=2),
            ctx_t[0:2].transpose([1, 0, 2]),
        )
        Yall = const.tile([128, BPC * Q], bf16, tag="Yall")
        nc.scalar.dma_start(
            Yall[:].rearrange("p (b q) -> p b q", b=BPC),
            q_t.transpose([1, 0, 2]),
        )
        w_row = const.tile([1, 3 * H], f32r, tag="w_row")
        nc.scalar.dma_start(w_row[:], w_t.unsqueeze(0).bitcast(f32r))

        ident = const.tile([128, 128], f32, tag="ident")
        masks.make_identity(nc, ident[:])
        identb = const.tile([128, 128], bf16, tag="identb")
        nc.vector.tensor_copy(identb[:], ident[:])
        identr = const.tile([128, 128], f32r, tag="identr")
        nc.gpsimd.tensor_copy(identr[:], ident[:])

        # w columns via K=1 PE matmuls against identity
        wps = ps_big.tile([128, C], f32, tag="big")
        nc.tensor.matmul(
            wps[:, 0:128], w_row[0:1, H : 2 * H], identr[0:1, 0:128],
            start=True, stop=True,
        )
        nc.tensor.matmul(
            wps[:, 128:256], w_row[0:1, 2 * H : 3 * H], identr[0:1, 0:128],
            start=True, stop=True,
        )
        wc = const.tile([128, 1], f32, tag="wc")
        wcq = const.tile([128, 1], f32, tag="wcq")
        nc.vector.tensor_copy(wc[:], wps[:, 0:1])
        nc.vector.tensor_copy(wcq[:], wps[:, 128:129])

        # all Z = wcq*Y + wc in one op; all Y^T via 8 PE transposes.
        # Emitted by prework() after front(0) so batch 0's score matmuls
        # and exp sit at the head of the PE/ACT queues.
        Zall = const.tile([128, BPC * Q], bf16, tag="Zall")
        YTall = const.tile([128, BPC * H], bf16, tag="YTall")

        # batch 0's Z must be written before front(0) is emitted — the
        # dependency tracker follows emission order and cannot see a
        # write that is emitted after its reader.
        nc.vector.tensor_scalar(
            Zall[:, 0:Q], Yall[:, 0:Q], wcq[:], wc[:], MULT, ADD
        )

        def prework():
            nc.vector.tensor_scalar(
                Zall[:, Q:], Yall[:, Q:], wcq[:], wc[:], MULT, ADD
            )
            ytp = ps_tp.tile([128, BPC * Q], bf16, tag="tp")
            for bb in range(BPC):
                nc.tensor.transpose(
                    ytp[:, bb * Q : (bb + 1) * Q],
                    Yall[:, bb * Q : (bb + 1) * Q],
                    identb[:],
                )
            nc.vector.tensor_copy(YTall[:], ytp[:])

        st = {}  # per-batch live tiles (3-stage pipeline keeps ~3 alive)

        pairs = {0: Xpair0}

        def load_pair(g):
            Xpair = sbx.tile([H, 2 * C], bf16, tag="Xp")
            nc.sync.dma_start(
                Xpair[:].rearrange("p (k c) -> p k c", k=2),
                ctx_t[2 * g : 2 * g + 2].transpose([1, 0, 2]),
            )
            pairs[g] = Xpair

        load_pair(1)  # pairs 0 and 1 in flight before the loop

        def front(b):
            state = st[b] = {}
            # prefetch two pairs ahead (sbx bufs=3 keeps 3 pairs alive)
            if b % 2 == 0 and b // 2 + 2 < BPC // 2:
                load_pair(b // 2 + 2)
            X = pairs[b // 2][:, (b % 2) * C : (b % 2 + 1) * C]

            # scores (2 matmuls into one 2-bank tile) + 2 exp chunks
            Sps = ps_big.tile([Q, C], f32, tag="big")
            P = sb.tile([Q, C], bf16, tag="P")
            dh = sb.tile([Q, 2], f32, tag="dh")
            Zb = Zall[:, b * Q : (b + 1) * Q]
            for j in range(2):
                nc.tensor.matmul(
                    Sps[:, j * 512 : (j + 1) * 512],
                    Zb,
                    X[:, j * 512 : (j + 1) * 512],
                    start=True,
                    stop=True,
                )
                nc.scalar.activation(
                    P[:, j * 512 : (j + 1) * 512],
                    Sps[:, j * 512 : (j + 1) * 512],
                    EXP,
                    accum_out=dh[:, j : j + 1],
                )
            state.update(X=X, P=P, dh=dh, b=b)

        def stats(b):
            # softmax stats; emitted late so the DVE queue serves the
            # data-ready evacuations first (these only gate A of batch b,
            # which runs next cycle)
            state = st[b]
            dh = state["dh"]
            dsum = sb.tile([Q, 1], f32, tag="dsum")
            nc.vector.tensor_add(dsum[:], dh[:, 0:1], dh[:, 1:2])
            rr = sb.tile([Q, 1], f32, tag="rr")
            nc.vector.reciprocal(rr[:], dsum[:])
            # YTs on ACT: a DVE tensor_scalar here enters 4x 2-port mode and
            # deadlocks against GpSimd's X*A for that instruction's duration
            YTs = sb.tile([Q, H], bf16, tag="YTs")
            nc.scalar.activation(
                YTs[:], YTall[:, b * H : (b + 1) * H], COPY, scale=rr[:]
            )
            state.update(rr=rr, YTs=YTs)

        def mid(b):
            state = st[b]
            X, P, YTs, rr = state["X"], state["P"], state["YTs"], state["rr"]
            # X^T / P^T: 8 transposes each into the single-bank bf16 tile,
            # evacuated with one copy (ACT for X^T, DVE for P^T).
            xtp = ps_tp.tile([128, C], bf16, tag="tp")
            for k in range(8):
                nc.tensor.transpose(
                    xtp[:, k * 128 : (k + 1) * 128],
                    X[:, k * 128 : (k + 1) * 128],
                    identb[:],
                )
            XT = sb.tile([128, C], bf16, tag="XT")
            nc.vector.tensor_copy(XT[:], xtp[:])

            ptp = ps_tp.tile([128, C], bf16, tag="tp")
            for k in range(8):
                nc.tensor.transpose(
                    ptp[:, k * 128 : (k + 1) * 128],
                    P[:, k * 128 : (k + 1) * 128],
                    identb[:],
                )
            PT = sb.tile([128, C], bf16, tag="PT")
            nc.vector.tensor_copy(PT[:], ptp[:])

            # tt = P @ X^T, accumulated over the 8 c-chunks
            tt = ps_tp.tile([Q, H], f32, tag="tp")
            for k in range(8):
                nc.tensor.matmul(
                    tt[:],
                    PT[:, k * 128 : (k + 1) * 128],
                    XT[:, k * 128 : (k + 1) * 128],
                    start=(k == 0),
                    stop=(k == 7),
                )
            # tts = tt * r^2, r^2 fused as a double scalar multiply
            tts = sb.tile([Q, H], bf16, tag="tts")
            nc.vector.tensor_scalar(tts[:], tt[:], rr[:], rr[:], MULT, MULT)

            # A matmuls into one 2-bank tile; evac halves on ACT and DVE.
            S3 = sb3.tile([H, 3 * C], bf16, tag="S3")
            Aps = ps_big.tile([H, C], f32, tag="big")
            for j in range(2):
                nc.tensor.matmul(
                    Aps[:, j * 512 : (j + 1) * 512],
                    YTs[:],
                    P[:, j * 512 : (j + 1) * 512],
                    start=True,
                    stop=True,
                )
            nc.scalar.activation(S3[:, 0:512], Aps[:, 0:512], COPY)
            nc.scalar.activation(S3[:, 512:1024], Aps[:, 512:1024], COPY)
            state.update(tts=tts, S3=S3)

        def back(b):
            state = st[b]
            X, P, tts, S3 = state["X"], state["P"], state["tts"], state["S3"]
            last = b == BPC - 1
            if last:
                nc.scalar.dma_start(out_t[b, 0], S3[:, 0:C])
                for j in range(2):
                    nc.gpsimd.tensor_mul(
                        S3[:, C + j * 512 : C + (j + 1) * 512],
                        X[:, j * 512 : (j + 1) * 512],
                        S3[:, j * 512 : (j + 1) * 512],
                    )
                    (nc.sync if j == 0 else nc.scalar).dma_start(
                        out_t[b, 1, :, j * 512 : (j + 1) * 512],
                        S3[:, C + j * 512 : C + (j + 1) * 512],
                    )
            else:
                nc.gpsimd.tensor_mul(S3[:, C : 2 * C], X[:], S3[:, 0:C])
            # B matmuls into one 2-bank tile; X*B straight from PSUM (one op)
            Bps = ps_big.tile([H, C], f32, tag="big")
            for j in range(2):
                nc.tensor.matmul(
                    Bps[:, j * 512 : (j + 1) * 512],
                    tts[:],
                    P[:, j * 512 : (j + 1) * 512],
                    start=True,
                    stop=True,
                )
            if last:
                for j in range(2):
                    nc.vector.tensor_mul(
                        S3[:, 2 * C + j * 512 : 2 * C + (j + 1) * 512],
                        X[:, j * 512 : (j + 1) * 512],
                        Bps[:, j * 512 : (j + 1) * 512],
                    )
                    (nc.sync if j == 0 else nc.scalar).dma_start(
                        out_t[b, 2, :, j * 512 : (j + 1) * 512],
                        S3[:, 2 * C + j * 512 : 2 * C + (j + 1) * 512],
                    )
            else:
                nc.vector.tensor_mul(S3[:, 2 * C : 3 * C], X[:], Bps[:])
                nc.sync.dma_start(
                    out_t[b].transpose([1, 0, 2]),
                    S3[:].rearrange("p (k c) -> p k c", k=3),
                )

        # 3-stage software pipeline: cycle i emits front(i) | mid(i-1) |
        # back(i-2). Each batch's serial dependency chain (exp -> stats ->
        # evacs -> tt -> B -> X*B -> DMA) gets ~2 cycles of latency budget,
        # so the steady-state cycle is set by engine throughput instead.
        for i in range(BPC + 2):
            if i < BPC:
                front(i)
            if i == 0:
                prework()
            if 0 <= i - 1 < BPC:
                mid(i - 1)
            if 0 <= i - 2 < BPC:
                back(i - 2)
                del st[i - 2]
            if i < BPC:
                stats(i)

    nc.compile()
    return nc


def kernel(context, question, w):
    global _NC, LAST_RESULTS
    import ml_dtypes

    from concourse import bass_utils

    if _NC is None:
        _NC = _build()

    context = np.ascontiguousarray(np.asarray(context), dtype=np.float32)
    question = np.ascontiguousarray(np.asarray(question), dtype=np.float32)
    w = np.ascontiguousarray(np.asarray(w), dtype=np.float32)
    ctx_bf = context.astype(ml_dtypes.bfloat16)
    q_bf = question.astype(ml_dtypes.bfloat16)

    in_maps = [
        {
            "context": ctx_bf[c * BPC : (c + 1) * BPC],
            "question": q_bf[c * BPC : (c + 1) * BPC],
            "w": w,
        }
        for c in range(NCORES)
    ]
    trace = bool(int(os.environ.get("KTRACE", "0")))
    LAST_RESULTS = bass_utils.run_bass_kernel_spmd(
        _NC, in_maps, core_ids=list(range(NCORES)), trace=trace
    )
    out = np.empty((B, 4 * H, C), dtype=np.float32)
    out[:, 0:H, :] = context
    for c in range(NCORES):
        blk = np.asarray(LAST_RESULTS.results[c]["out"]).astype(np.float32)
        out[c * BPC : (c + 1) * BPC, H:, :] = blk.reshape(BPC, 3 * H, C)
    return out
